# revision 1
# baseline (speedup 1.0000x reference)
"""Bass/Trainium2 kernel for nn_AttentionHead (B=4, C=D=64, H=W=64).

Sharding: 8 cores = 4 batches x 2 query-halves. Each core holds the full
x for its batch (keys/values need all 4096 positions) and computes
attention for 2048 query positions. Per-core inputs are column-rotated so
every core's query block is columns [0, 2048) of its own x — the program
is identical across cores (SPMD), only data differs. Softmax over keys is
permutation-invariant, so rotating the key order is exact.

On-device math (per core, N=4096 keys, NH=2048 queries):
  GroupNorm(num_groups=C) is affine per (batch, channel): xn = s*x + t with
  s = gn_w*rsqrt(var+eps), t = gn_b - mu*s. Folded into the projections via
  an all-ones row 64 of x_aug (biases become contractions).

  fp8 attention core: Q,K,V' are quantized to fp8-e4m3 (output noise is
  dominated by the out-proj bias; measured end-to-end rel err ~5e-3).
  Scores are computed transposed, S^T[m,n] = K[:,m]^T Q[:,n], as fp8
  DoubleRow matmuls whose second row-group reads an all-zero plane — the
  cost model charges 0.5 cyc/col for DR regardless, so K=64 contraction
  runs at 2x effective rate. exp(S^T/8) is produced as e4m3 BYTES three
  ways in parallel: ACT runs exact Exp with fp8 output; DVE and GPSIMD run
  a Schraudolph affine (trunc(s*8*log2e*SCALE + 56.5) into int8 IS the
  e4m3 encoding of exp, max rel err ~11%, softmax-normalized away).
  A V' ones-column gives the softmax denominator through the same AV
  matmuls (fp8 DoubleRow over adjacent key-tile pairs, accumulating in
  PSUM), and the final division is folded past the output projection:
  y = (wo@O + bo*denom) * (1/denom).
"""

import sys

sys.path.insert(0, "/opt/trn_rl_repo")

from contextlib import ExitStack

import numpy as np

import concourse.bass as bass
import concourse.tile as tile
from concourse import mybir
from concourse.bass import ts
from concourse.bass_utils import run_bass_kernel_spmd

# ---------------------------------------------------------------------------
# Workaround: this toolchain's walrus accepts at most ONE semaphore wait per
# instruction, but Tile's scheduler (and its tail drain) can attach several.
# Post-process every block, hoisting excess waits onto InstEventSemaphore
# carriers inserted immediately before the offending instruction on the same
# engine (engines execute their streams in order => semantically identical).
from concourse.vector_clock import ScopedClock as _ScopedClock
from concourse.bass import _bass_rust as _br


def _split_multiwait_instructions(nc, h0):
    cur_bb = nc.cur_bb.bb
    for f in nc.m.functions:
        for bb in f.blocks:
            insts = list(bb.instructions)
            out = []
            changed = False
            for ins in insts:
                si = ins.sync_info
                if si is not None:
                    waits = list(si.on_wait)
                    if len(waits) > 1:
                        for w in waits[:-1]:
                            carrier = nc.engines[ins.engine].wait_ge(h0, 0).ins
                            lst = list(cur_bb.instructions)
                            assert lst and lst[-1].name == carrier.name
                            lst.pop()
                            cur_bb.instructions = lst
                            carrier.sync_info.on_wait = [w]
                            out.append(carrier)
                        si.on_wait = [waits[-1]]
                        changed = True
                out.append(ins)
            if changed:
                bb.instructions = out


def _patched_drain_and_barrier(self, tick_clock, wait_clock):
    nc = self.nc
    assert self.sems is not None
    h0 = next(iter(self.sems.allocated().values()), None)
    if h0 is not None:
        _split_multiwait_instructions(nc, h0)

    drain_inst = nc.sync.drain()
    wait_clock.add_sem_waits(
        drain_inst.ins, _ScopedClock({None: tick_clock.global_clock})
    )
    si = drain_inst.ins.sync_info
    if si is not None:
        waits = list(si.on_wait)
        if len(waits) > 1:
            si.on_wait = [waits[0]]
            for w in waits[1:]:
                d2 = nc.sync.drain()
                _br.wait_op(d2.ins, h0, 0, "sem-ge", False)
                d2.ins.sync_info.on_wait = [w]

    nc.all_engine_barrier()
    popped = nc._tile_sem_poison_stack.pop()
    assert popped is self._sem_poison
    nc.clear_and_free_semaphores(list(self.sems.allocated().values()))
    nc.all_engine_barrier()


tile.TileContext._drain_and_barrier = _patched_drain_and_barrier
# ---------------------------------------------------------------------------

B, C, D, H, W = 4, 64, 64, 64, 64
N = H * W  # 4096 spatial positions (keys)
NCORES = 8
NH = N // 2  # 2048 queries per core
NT = 512  # query-tile width
MT = 128  # key-tile height
NJ = NH // NT  # 4 query tiles
NM = N // MT  # 32 key tiles
NP = NM // 2  # 16 key-tile pairs (one fp8-DR AV matmul each)
EPS = 1e-5
SCALE = 1.0 / np.sqrt(np.float32(D))
# Schraudolph constants: int8 bits of e4m3(exp(s*SCALE)) ~= trunc(s*A8 + B8)
A8 = float(8.0 * np.log2(np.e) * SCALE)
B8 = 56.5  # e4m3 bias 7 -> +56; +0.5 compensates trunc-toward-zero
F32 = mybir.dt.float32
F32R = mybir.dt.float32r  # TF32-like fast-fp32 matmul (1 cyc/col, mov>=256)
F8 = mybir.dt.float8e4  # ml_dtypes.float8_e4m3 (bias 7)
I8 = mybir.dt.int8
DR = mybir.MatmulPerfMode.DoubleRow

_cache = {}


def _pattern(nA, nD, nP):
    """Evenly interleave nA 'A', nD 'D', nP 'P' engine slots."""
    n = nA + nD + nP
    acc = {"A": [0.0, nA], "D": [0.0, nD], "P": [0.0, nP]}
    out = []
    for _ in range(n):
        for k in acc:
            acc[k][0] += acc[k][1]
        k = max(acc, key=lambda k: acc[k][0])
        acc[k][0] -= n
        out.append(k)
    return out


# exp-group engine split per j-tile (16 groups of [128, 1024] each)
EXP_PAT = [
    _pattern(6, 4, 6),
    _pattern(6, 4, 6),
    _pattern(6, 4, 6),
    _pattern(6, 4, 6),
]


def _build_nc(reps=1):
    nc = bass.Bass()
    x_d = nc.declare_dram_parameter("x", [C, N], F32R, isOutput=False)
    # cols [0:64] wqT + bq_row@64, [64:128] wkT + bk_row@64,
    # [128:192] wvT + bv_row@64, [192:256] woT + bo@64,
    # col 256 gn_weight (rows 0:64), col 257 gn_bias
    wp_d = nc.declare_dram_parameter("wpack", [C + 1, 258], F32, isOutput=False)
    out_d = nc.declare_dram_parameter("out", [D, NH], F32, isOutput=True)

    with tile.TileContext(nc) as tc, ExitStack() as ctx:
        consts = ctx.enter_context(tc.tile_pool(name="consts", bufs=1))
        big = ctx.enter_context(tc.tile_pool(name="big", bufs=1))
        exps = ctx.enter_context(tc.tile_pool(name="exps", bufs=2))
        outp = ctx.enter_context(tc.tile_pool(name="outp", bufs=3))
        ps_s = ctx.enter_context(tc.tile_pool(name="ps_s", bufs=3, space="PSUM"))
        ps_x = ctx.enter_context(tc.tile_pool(name="ps_x", bufs=1, space="PSUM"))
        ps_a = ctx.enter_context(tc.tile_pool(name="ps_a", bufs=1, space="PSUM"))

        with nc.allow_low_precision(reason="fp8 attention core"):
            for _rep in range(reps):
                _emit_body(nc, tc, consts, big, exps, outp, ps_s, ps_x, ps_a,
                           x_d, wp_d, out_d)

    return nc


def _emit_body(nc, tc, consts, big, exps, outp, ps_s, ps_x, ps_a, x_d, wp_d,
               out_d):
    # warm the ACT table set (natural_log_exp_and_others: ln/exp/copy) under
    # the input DMAs so the ~1.3us load is off the critical chain
    warm = consts.tile([1, 1], F32)
    nc.gpsimd.memset(warm, 0.0)

    # ---- load weights / params: one packed DMA ---------------------
    dma = nc.default_dma_engine
    wp = consts.tile([C + 1, 258], F32)
    nc.gpsimd.dma_start(out=wp, in_=wp_d[:])
    wqT = wp[0:C, 0:D]
    wkT = wp[0:C, D : 2 * D]
    wvT = wp[0:C, 2 * D : 3 * D]
    woTa = consts.tile([C + 1, D], F32R)
    nc.vector.tensor_copy(out=woTa, in_=wp[:, 3 * D : 4 * D])
    bq_row = wp[C : C + 1, 0:D]
    bk_row = wp[C : C + 1, D : 2 * D]
    bv_row = wp[C : C + 1, 2 * D : 3 * D]
    gnw = wp[0:C, 256:257]
    gnb = wp[0:C, 257:258]

    # ---- fp8 Q|K buffer with an all-zero second DR group ------------
    # qk8[:, 0, 0:NH] = Q^ (fp8), qk8[:, 0, NH:NH+N] = K^ (fp8),
    # qk8[:, 1, :] = zeros (the dead half of every DoubleRow matmul).
    # The zero plane is filled by SBUF->SBUF DMAs from a small memset
    # block (engine-time free, DMA engines are idle).
    QK = NH + N  # 6144
    qk8 = big.tile([C, 2, QK], F8)
    zblk = consts.tile([C, 768], F8)
    nc.gpsimd.memset(zblk, 0.0)

    # PE p-state warmup: the tensor engine needs ~3us of continuous work
    # to reach full clock. Chain dummy fp8 matmuls over the zero block
    # while x streams in, so the projections and first scores run at
    # full speed. The chain is long enough that real PE work queues up
    # behind it with no idle gap (idle resets the ramp).
    wps = ps_x.tile([MT, NT], F32, tag="x")
    for _ in range(20):
        nc.tensor.matmul(
            wps, zblk[:, 0:MT], zblk[:, 0:NT], start=True, stop=True,
            skip_group_check=True,
        )

    # ---- x with an all-ones row 64 ---------------------------------
    # chunked DMA so bn_stats can start on chunk 0 while later chunks
    # are still in flight
    x_aug = big.tile([C + 1, N], F32R)
    x_f32 = x_aug.bitcast(F32)
    stats = consts.tile([C, N // 512, 6], F32)
    dma_engines = [nc.sync, nc.gpsimd]
    for q in range(8):
        dma_engines[q % 2].dma_start(
            out=x_aug[0:C, ts(q, 512)], in_=x_d[:, ts(q, 512)]
        )
        nc.vector.bn_stats(out=stats[:, q, :], in_=x_f32[0:C, ts(q, 512)])
    nc.gpsimd.memset(x_f32[C : C + 1, 0 : N // 2], 1.0)
    nc.gpsimd.memset(x_f32[C : C + 1, N // 2 : N], 1.0)
    x_r = x_aug
    nc.scalar.activation(
        out=warm, in_=warm, func=mybir.ActivationFunctionType.Exp
    )
    for z in range(8):
        eng = nc.sync if z % 2 == 0 else nc.scalar
        eng.dma_start(out=qk8[:, 1, ts(z, 768)], in_=zblk)
    ones_col = consts.tile([1, D], F32)
    nc.gpsimd.memset(ones_col, 1.0)
    mv = consts.tile([C, 2], F32)
    nc.vector.bn_aggr(out=mv, in_=stats)

    # rs = (var+eps)^-0.5 via Ln/Exp (stays in the one ACT table set)
    epst = consts.tile([C, 1], F32)
    nc.gpsimd.memset(epst, EPS)
    lnv = consts.tile([C, 1], F32)
    nc.scalar.activation(
        out=lnv, in_=mv[:, 1:2], func=mybir.ActivationFunctionType.Ln, bias=epst
    )
    rs = consts.tile([C, 1], F32)
    nc.scalar.activation(
        out=rs, in_=lnv, func=mybir.ActivationFunctionType.Exp, scale=-0.5
    )
    s_vec = consts.tile([C, 1], F32)
    nc.vector.tensor_mul(out=s_vec, in0=rs, in1=gnw)
    mus = consts.tile([C, 1], F32)
    nc.vector.tensor_mul(out=mus, in0=mv[:, 0:1], in1=s_vec)
    t_vec = consts.tile([C, 1], F32)
    nc.vector.tensor_sub(out=t_vec, in0=gnb, in1=mus)

    # ---- augmented projection weights ------------------------------
    # what_* rows 0..63 = w^T * s (per-channel), row 64 = (w@t + b)^T.
    what_q = consts.tile([C + 1, D], F32R)
    what_k = consts.tile([C + 1, D], F32R)
    what_v = consts.tile([C + 1, D + 1], F32)
    nc.gpsimd.memset(what_v[:, D : D + 1], 0.0)
    nc.vector.tensor_scalar_mul(out=what_q[0:C, :], in0=wqT, scalar1=s_vec)
    nc.vector.tensor_scalar_mul(out=what_k[0:C, :], in0=wkT, scalar1=s_vec)
    nc.vector.tensor_scalar_mul(out=what_v[0:C, 0:D], in0=wvT, scalar1=s_vec)
    for whT, wT, b_row, c0, c1 in (
        (what_q, wqT, bq_row, 0, D),
        (what_k, wkT, bk_row, 0, D),
        (what_v, wvT, bv_row, 0, D),
    ):
        r_ps = ps_x.tile([MT, NT], F32, tag="x")
        nc.tensor.matmul(r_ps[0:1, 0:D], t_vec, wT, start=True, stop=True)
        nc.vector.tensor_add(
            out=whT[C : C + 1, c0:c1], in0=r_ps[0:1, 0:D], in1=b_row
        )
    nc.gpsimd.memset(what_v[C : C + 1, D : D + 1], 1.0)
    what_q_r = what_q
    what_k_r = what_k

    # ---- projections ------------------------------------------------
    # K^: 8 chunks of 512 keys -> fp8 converts in 4 groups of 1024
    # (engines: 2x DVE, 2x Pool). Q^: 4 j-tiles -> 2 ACT converts.
    # V'^T: 32 tiles of [128 keys, 65ch] (64 ch + ones col) -> v8 fp8.
    VP = MT  # dual-fp8 ldweights wants full 128-column weight groups
    v8 = big.tile([MT, NM, VP], F8)
    nc.gpsimd.memset(v8[:, :, D + 1 : VP], 0.0)

    def emit_k(g):  # g in 0..3, covers key chunks 2g, 2g+1
        p = ps_s.tile([C, 2, NT], F32, tag="s")
        for i in range(2):
            nc.tensor.matmul(
                p[:, i, :], what_k_r, x_r[:, ts(2 * g + i, NT)],
                start=True, stop=True,
            )
        if g % 2 == 0:
            nc.vector.tensor_copy(
                out=qk8[:, 0, NH + 1024 * g : NH + 1024 * (g + 1)], in_=p
            )
        else:
            nc.scalar.activation(
                out=qk8[:, 0, NH + 1024 * g : NH + 1024 * (g + 1)], in_=p,
                func=mybir.ActivationFunctionType.Copy,
            )

    def emit_q(g):  # g in 0..1, covers query tiles 2g, 2g+1
        p = ps_s.tile([C, 2, NT], F32, tag="s")
        for i in range(2):
            nc.tensor.matmul(
                p[:, i, :], what_q_r, x_r[:, ts(2 * g + i, NT)],
                start=True, stop=True,
            )
            nc.scalar.activation(
                out=qk8[:, 0, ts(2 * g + i, NT)], in_=p[:, i, :],
                func=mybir.ActivationFunctionType.Copy,
            )

    def emit_vt(m0, cnt):  # cnt <= 14 tiles of [128, 65]; 7 per psum bank
        p = ps_s.tile([MT, 2, NT], F32, tag="s")
        for k in range(cnt):
            b, off = divmod(k, 7)
            nc.tensor.matmul(
                p[:, b, off * (D + 1) : (off + 1) * (D + 1)],
                x_f32[:, ts(m0 + k, MT)],
                what_v,
                start=True, stop=True,
            )
        for b in range((cnt + 6) // 7):
            bc = min(7, cnt - 7 * b)
            nc.scalar.activation(
                out=v8[:, m0 + 7 * b : m0 + 7 * b + bc, 0 : D + 1],
                in_=p[:, b, 0 : bc * (D + 1)],
                func=mybir.ActivationFunctionType.Copy,
            )

    # ---- attention core ---------------------------------------------
    def emit_unit(j, m0, sz, eng, uid):
        # scores + exp for key-tiles m0..m0+sz-1 (sz in {1, 2}). 2-tile
        # units use the 3 double-bank score slots, 1-tile units a 4th
        # single-bank slot, so 4 slot-holders > 3 engines and the psum
        # ring never paces the loop.
        if sz == 2:
            sp = ps_s.tile([MT, 2, NT], F32, tag="s", name=f"sp_{j}_{uid}")
            sin = sp
        else:
            sp1 = ps_x.tile([MT, NT], F32, tag="x", name=f"sx_{j}_{uid}")
            sin = sp1
        for i in range(sz):
            m = m0 + i
            o = sp[:, i, :] if sz == 2 else sp1
            nc.tensor.matmul(
                o,
                qk8[:, :, NH + MT * m : NH + MT * (m + 1)],
                qk8[:, :, ts(j, NT)],
                start=True, stop=True,
                perf_mode=DR,
            )
        if eng == "A":
            nc.scalar.activation(
                out=e8j_cur[0][:, m0 : m0 + sz, :], in_=sin,
                func=mybir.ActivationFunctionType.Exp, scale=float(SCALE),
            )
        else:
            e = nc.vector if eng == "D" else nc.gpsimd
            e.tensor_scalar(
                out=e8j_cur[0][:, m0 : m0 + sz, :].bitcast(I8), in0=sin,
                scalar1=A8, scalar2=B8,
                op0=mybir.AluOpType.mult, op1=mybir.AluOpType.add,
            )

    def emit_av(j, t, e8j, o_ps):
        nc.tensor.matmul(
            o_ps,
            v8[:, 2 * t : 2 * t + 2, :],
            e8j[:, 2 * t : 2 * t + 2, :],
            start=(t == 0), stop=(t == NP - 1),
            perf_mode=DR,
        )

    def emit_epilogue(j, o_ps):
        # y = (woTa.T @ O) * (1/denom); rb broadcasts 1/denom to 64
        # partitions via a PE ones-matmul; the final mul reads both z and
        # rb straight from PSUM. z reuses o_ps's pool (ring of 1).
        rec = outp.tile([1, NT], F32, tag="rec")
        nc.vector.reciprocal(out=rec, in_=o_ps[D : D + 1, :])
        o_sb = outp.tile([D + 1, NT], F32R, tag="osb")
        nc.vector.tensor_copy(out=o_sb, in_=o_ps[0 : D + 1, :])
        # z on partitions 0:64 and the 1/denom broadcast on 64:128 of the
        # SAME psum bank (PE ones-matmul with output partition offset 64)
        zrb = ps_x.tile([MT, NT], F32, tag="x", name=f"zrb_{j}")
        nc.tensor.matmul(zrb[D:MT, :], ones_col, rec, start=True, stop=True,
                         skip_group_check=True)
        nc.tensor.matmul(zrb[0:D, :], woTa, o_sb, start=True, stop=True,
                         skip_group_check=True)
        rb_sb = outp.tile([D, NT], F32, tag="rb")
        nc.scalar.activation(out=rb_sb, in_=zrb[D:MT, :],
                             func=mybir.ActivationFunctionType.Copy)
        y_sb = outp.tile([D, NT], F32, tag="y")
        nc.vector.tensor_mul(out=y_sb, in0=zrb[0:D, :], in1=rb_sb)
        dma.dma_start(out=out_d[:, ts(j, NT)], in_=y_sb)

    # ---- schedule ----------------------------------------------------
    # Unit plan per j: sizes [2,2,2,1]x4 + [2,2] covering the 32 key
    # tiles. Engines assigned by tile-count targets (A/D/P weights).
    UNITS = [2] * 16
    TGTS = [
        {"A": 18, "D": 14, "P": 0},  # gpsimd cannot read PSUM on HW
        {"A": 18, "D": 14, "P": 0},
        {"A": 18, "D": 14, "P": 0},
        {"A": 18, "D": 14, "P": 0},
    ]

    def unit_engines(tgt):
        assigned = {k: 0.0 for k in tgt}
        done = 0
        out = []
        for sz in UNITS:
            done += sz
            k = max(tgt, key=lambda k: tgt[k] * done / 32.0 - assigned[k]
                    - (0.01 if k == "D" else 0))
            assigned[k] += sz
            out.append(k)
        return out

    UENGS = [unit_engines(t) for t in TGTS]

    # Minimal prefix for j0 (K chunk-pair 0 -> key tiles 0..7, Q tiles
    # 0..1, V' tiles 0..13 -> AV pairs 0..6); the rest of the projections
    # are injected into j0's loop just before they are needed.
    emit_k(0)
    emit_q(0)
    emit_vt(0, 14)
    inject = {
        (0, 0): lambda: emit_k(1),
        (0, 4): lambda: emit_k(2),
        (0, 6): lambda: emit_vt(14, 14),
        (0, 8): lambda: emit_k(3),
        (0, 10): lambda: emit_q(1),
        (0, 12): lambda: emit_vt(28, 4),
    }

    e8j_cur = [None]
    pending = [None]  # (j, o_ps) of the epilogue not yet emitted
    for j in range(NJ):
        e8j = exps.tile([MT, NM, NT], F8, tag="e")
        e8j_cur[0] = e8j
        o_ps = ps_a.tile([MT, NT], F32, tag="a")
        m0 = 0
        mdone = []  # cumulative tiles after each unit
        av_next = 0
        for u, sz in enumerate(UNITS):
            emit_unit(j, m0, sz, UENGS[j][u], u)
            m0 += sz
            mdone.append(m0)
            fn = inject.get((j, u))
            if fn is not None:
                fn()
            if u == 2 and pending[0] is not None:
                emit_epilogue(*pending[0])
                pending[0] = None
            # AV pairs whose exps were emitted >= 2 units ago
            avail = mdone[u - 2] if u >= 2 else 0
            while av_next < NP and 2 * (av_next + 1) <= avail:
                emit_av(j, av_next, e8j, o_ps)
                av_next += 1
        while av_next < NP:
            emit_av(j, av_next, e8j, o_ps)
            av_next += 1
        pending[0] = (j, o_ps)
    emit_epilogue(*pending[0])


def _get_nc():
    if "nc" not in _cache:
        _cache["nc"] = _build_nc()
    return _cache["nc"]


class _Runner:
    """Cached SPMD executor: builds the shard_map'd jit once so repeat calls
    skip retracing (run_bass_via_pjrt rebuilds its jit on every call)."""

    def __init__(self, nc, n_cores):
        import jax
        from jax.sharding import Mesh, PartitionSpec
        from jax.experimental.shard_map import shard_map
        from concourse import bass2jax
        from concourse import mybir as _mb

        bass2jax.install_neuronx_cc_hook()
        partition_name = (
            nc.partition_id_tensor.name if nc.partition_id_tensor else None
        )
        in_names, out_names, out_avals, zero_outs = [], [], [], []
        for alloc in nc.m.functions[0].allocations:
            if not isinstance(alloc, _mb.MemoryLocationSet):
                continue
            name = alloc.memorylocations[0].name
            if alloc.kind == "ExternalInput":
                if name != partition_name:
                    in_names.append(name)
            elif alloc.kind == "ExternalOutput":
                out_names.append(name)
                shape = tuple(alloc.tensor_shape)
                dtype = _mb.dt.np(alloc.dtype)
                out_avals.append(jax.core.ShapedArray(shape, dtype))
                zero_outs.append(np.zeros(shape, dtype))
        self.in_names = list(in_names)
        self.out_names = list(out_names)
        self.out_avals = out_avals
        self.zero_outs = zero_outs
        n_params = len(in_names)
        all_in_names = in_names + out_names
        if partition_name is not None:
            all_in_names = all_in_names + [partition_name]

        def _body(*args):
            operands = list(args)
            if partition_name is not None:
                operands.append(bass2jax.partition_id_tensor())
            outs = bass2jax._bass_exec_p.bind(
                *operands,
                out_avals=tuple(out_avals),
                in_names=tuple(all_in_names),
                out_names=tuple(out_names),
                lowering_input_output_aliases=(),
                sim_require_finite=True,
                sim_require_nnan=True,
                nc=nc,
            )
            return tuple(outs)

        devices = jax.devices()[:n_cores]
        mesh = Mesh(np.asarray(devices), ("core",))
        n_outs = len(out_names)
        self.n_cores = n_cores
        self.fn = jax.jit(
            shard_map(
                _body,
                mesh=mesh,
                in_specs=(PartitionSpec("core"),) * (n_params + n_outs),
                out_specs=(PartitionSpec("core"),) * n_outs,
                check_rep=False,
            ),
            keep_unused=True,
        )

    def concat_inputs(self, in_maps):
        cat = [
            np.concatenate([m[name] for m in in_maps], axis=0)
            for name in self.in_names
        ]
        cat += [
            np.zeros((self.n_cores * z.shape[0], *z.shape[1:]), z.dtype)
            for z in self.zero_outs
        ]
        return cat

    def __call__(self, concat_in):
        return self.fn(*concat_in)

    def run(self, in_maps):
        import jax

        out_arrs = jax.block_until_ready(self(self.concat_inputs(in_maps)))
        return [
            {
                name: np.asarray(out_arrs[i]).reshape(
                    self.n_cores, *self.out_avals[i].shape
                )[c]
                for i, name in enumerate(self.out_names)
            }
            for c in range(self.n_cores)
        ]


def _get_runner():
    if "runner" not in _cache:
        _cache["runner"] = _Runner(_get_nc(), NCORES)
    return _cache["runner"]


def _make_in_maps(x, gn_weight, gn_bias, wq, bq, wk, bk, wv, bv, wo, bo):
    f = lambda a: np.ascontiguousarray(np.asarray(a, dtype=np.float32))
    x = f(x)
    wpack = np.zeros((C + 1, 258), dtype=np.float32)
    for i, (w, b) in enumerate(((wq, bq), (wk, bk), (wv, bv), (wo, bo))):
        wpack[0:C, i * D : (i + 1) * D] = f(w).T
        wpack[C, i * D : (i + 1) * D] = f(b)
    wpack[0:C, 256] = f(gn_weight)
    wpack[0:C, 257] = f(gn_bias)
    shared = {"wpack": wpack}
    in_maps = []
    for i in range(NCORES):
        b, h = divmod(i, 2)
        xb = x[b].reshape(C, N)
        if h:
            xb = np.concatenate([xb[:, NH:], xb[:, :NH]], axis=1)
        in_maps.append({"x": np.ascontiguousarray(xb), **shared})
    return in_maps


def kernel(x, gn_weight, gn_bias, wq, bq, wk, bk, wv, bv, wo, bo):
    in_maps = _make_in_maps(x, gn_weight, gn_bias, wq, bq, wk, bk, wv, bv, wo, bo)
    results = _get_runner().run(in_maps)
    out = np.empty((B, D, N), dtype=np.float32)
    for i in range(NCORES):
        b, h = divmod(i, 2)
        out[b, :, h * NH : (h + 1) * NH] = results[i]["out"]
    return out.reshape(B, D, H, W)

